# revision 61
# baseline (speedup 1.0000x reference)
"""Trainium2 Bass kernel for nn_AttentionLayer (pooling attention).

Reference computation (S=2048, B=64, H=512):
    r      = (mask * sent).transpose(1,0,2)        # (B, S, H)
    WY     = r @ W
    WR     = mean_sent @ W_h
    M      = tanh(WY + WR[:, None, :])
    scores = M @ context                            # (B, S)
    alpha  = softmax(scores, axis=1)
    out    = sum_s alpha * r                        # (B, H)

Sharding: data-parallel over B across 8 cores (8 batches/core); W, W_h,
context replicated.

Per-core dataflow (heavy matmuls in bf16, fp32 accumulation):
  - one 2 MB SWDGE DMA per batch loads sent[b] HBM->SBUF with inline
    fp32->bf16 cast into natural-layout tiles [s_part, (chunk, h)] that
    stay resident until the batch's final reduction (~1.5 batches).
  - one xbar DMA-transpose per (b, stile) produces all 16 r^T 128x128
    blocks (h-on-partitions) in a single instruction (3D out AP).
  - WY^T[k, s] = sum_h W[h,k] r^T[h,s] accumulated in PSUM (W stationary,
    bf16); tanh applied by ScalarE directly from PSUM with per-partition
    bias WR^T[k, b] (so the WR add is free).
  - scores[s] = sum_k ctx[k] tanh[k, s]: the k-chunks are pre-reduced on
    DVE (ctx as per-partition tensor_scalar weight), then one
    ones-column matmul sums over partitions.
  - softmax over s WITHOUT max subtraction: |scores| <= ||ctx||_1 (tanh
    is bounded by 1), ~23 worst-case for this problem's context scaling,
    so exp cannot overflow fp32.  1/sumexp is folded into alpha.
  - alpha^T (s-on-partitions, needed as the stationary operand of the
    final matmul) is built without any DMA: a K=1 ones-matmul broadcasts
    the alpha row to all partitions, an identity-mask multiply + per-128
    window reduce_sum extracts alpha^T[p, c] = alpha[c*128+p].
  - out[b, :] = sum_c alpha^T[:, c]^T @ r_nat[c] accumulated in PSUM.
  - the final phase of batch b is emitted inside batch b+1's WY phase so
    no engine stalls on the softmax chain.

Quirks of this container's toolchain that shaped the implementation:
  - built on bacc.Bacc (not bass.Bass): Bacc.compile() runs
    generate_event_semaphores, which splits multi-semaphore sync waits
    (walrus here rejects >1 sync wait on most instructions).
  - Tile serializes any copy-mode DMA against xbar-transpose DMAs (HW
    deadlock workaround), so the output store is fenced behind a
    no_sync_barrier + one junk DMA that absorbs the mode transition.
  - tensor_tensor_reduce is not supported by this walrus (ISA wrong
    length), hence the identity-mask + reduce_sum alpha extraction.
"""

import os
import numpy as np

import concourse.bass as bass
import concourse.mybir as mybir
import concourse.tile as tile
from concourse import bacc, bass_utils

FP32 = mybir.dt.float32
BF16 = mybir.dt.bfloat16

H = 512
S = 2048
B = 64
NCORES = 8
BPC = B // NCORES  # batches per core

HC = H // 128      # h chunks of 128 (contraction)
KC = H // 128      # k chunks of 128 (output dim of W)

_cache = {}


def _build_nc(bpc=BPC, s=S):
    st_n = s // 512
    nc = bacc.Bacc(None, target_bir_lowering=False)
    sent = nc.dram_tensor("sent", [bpc, s, H], FP32, kind="ExternalInput")
    mean = nc.dram_tensor("mean_sent", [bpc, H], FP32, kind="ExternalInput")
    w = nc.dram_tensor("w", [H, H], FP32, kind="ExternalInput")
    wh = nc.dram_tensor("wh", [H, H], FP32, kind="ExternalInput")
    ctxv = nc.dram_tensor("ctxv", [H], FP32, kind="ExternalInput")
    out = nc.dram_tensor("out", [bpc, H], FP32, kind="ExternalOutput")

    with tile.TileContext(nc) as tc:
        with tc.tile_pool(name="singles", bufs=1) as singles, \
             tc.tile_pool(name="keep", bufs=1) as keep, \
             tc.tile_pool(name="rt", bufs=3) as rt_pool, \
             tc.tile_pool(name="th", bufs=2) as th_pool, \
             tc.tile_pool(name="sm", bufs=2) as sm_pool, \
             tc.tile_pool(name="wy", bufs=3, space="PSUM") as wy_pool, \
             tc.tile_pool(name="scp", bufs=2, space="PSUM") as sc_pool, \
             tc.tile_pool(name="abp", bufs=1, space="PSUM") as ab_pool:

            # issue batch 0's loads first so the pipeline fills during prep;
            # its first s-tile gets a separate small tile so the first
            # transpose starts after ~0.5 MB instead of the full 2 MB
            rn0a = keep.tile([128, 4 * H], BF16, tag="rn0a", bufs=1, name="rn0a")
            nc.gpsimd.dma_start(
                out=rn0a.rearrange("p (t h) -> p t h", t=4),
                in_=sent[0, 0: 512].rearrange("(t p) h -> p t h", p=128),
            )
            rnat0 = keep.tile(
                [128, 4 * st_n * H], BF16, tag="rn", bufs=3, name="rn0"
            )
            if st_n > 1:
                nc.gpsimd.dma_start(
                    out=rnat0.rearrange("p (t h) -> p t h", t=4 * st_n)[:, 4:, :],
                    in_=sent[0, 512: s].rearrange("(t p) h -> p t h", p=128),
                )

            # ---- constants / small precompute ----
            # W as bf16, [h_part, (hc k)] : w_bf[p, hc*H + k] = W[hc*128+p, k]
            w_bf = singles.tile([128, HC * H], BF16, tag="w_bf")
            nc.gpsimd.dma_start(
                out=w_bf.rearrange("p (hc k) -> p hc k", hc=HC),
                in_=w.rearrange("(hc p) k -> p hc k", p=128),
            )
            # W_h fp32 same layout (used for WR precompute, stays fp32)
            wh_sb = singles.tile([128, HC * H], FP32, tag="wh_sb")
            nc.sync.dma_start(
                out=wh_sb.rearrange("p (hc k) -> p hc k", hc=HC),
                in_=wh.rearrange("(hc p) k -> p hc k", p=128),
            )
            # mean transposed: meanT[p, hc*bpc + b] = mean[b, hc*128+p]
            meanT = singles.tile([128, HC * bpc], FP32, tag="meanT")
            for hc in range(HC):
                nc.sync.dma_start(
                    out=meanT[:, hc * bpc: (hc + 1) * bpc],
                    in_=mean[:, hc * 128: (hc + 1) * 128].rearrange("b p -> p b"),
                )
            # context transposed bf16: ctxT[p, c] = ctx[c*128+p]
            ctxT = singles.tile([128, KC], BF16, tag="ctxT")
            nc.gpsimd.dma_start(
                out=ctxT, in_=ctxv.rearrange("(c p) -> p c", p=128)
            )
            # ones row for the alpha partition-broadcast matmul (K=1)
            ones_row = singles.tile([1, 128], BF16, tag="ones_row")
            nc.vector.memset(ones_row, 1.0)
            # ones column for partition-sum matmuls
            ones_col = singles.tile([128, 1], BF16, tag="ones_col")
            nc.vector.memset(ones_col, 1.0)
            # fp32 copy of ctx^T for per-partition tensor_scalar weighting
            ctxT_f32 = singles.tile([128, KC], FP32, tag="ctxT_f32")
            nc.vector.tensor_copy(ctxT_f32, ctxT)

            # WR^T[k, b] = sum_h W_h[h, k] * mean[b, h]  (fp32)
            wrT = singles.tile([128, KC * bpc], FP32, tag="wrT")
            for kc in range(KC):
                wr_ps = ab_pool.tile([128, bpc], FP32, tag="fin_ps", bufs=1)
                for hc in range(HC):
                    nc.tensor.matmul(
                        wr_ps,
                        lhsT=wh_sb[:, hc * H + kc * 128: hc * H + (kc + 1) * 128],
                        rhs=meanT[:, hc * bpc: (hc + 1) * bpc],
                        start=(hc == 0),
                        stop=(hc == HC - 1),
                    )
                nc.vector.tensor_copy(wrT[:, kc * bpc: (kc + 1) * bpc], wr_ps)

            # ACT wait-absorber: a dummy op reading the last DVE-written wrT
            # chunk so per-kc tanh activations only need their PE wait.
            act_scratch = singles.tile([128, bpc], FP32, tag="act_scratch")
            nc.scalar.activation(
                act_scratch,
                wrT[:, (KC - 1) * bpc:],
                mybir.ActivationFunctionType.Copy,
            )

            # identity mask (bf16) for extracting alpha^T from the
            # partition-broadcast alpha rows
            from concourse.masks import make_identity
            ident_bf = singles.tile([128, 128], BF16, tag="ident_bf")
            make_identity(nc, ident_bf)

            # bf16 natural-layout r tiles, kept ~1.5 batches (the final
            # reduction for batch b runs during batch b+1), so a rotating
            # pool of 3*st_n slots suffices;
            # tile[p, sc*H + h] = sent[b, st*512 + sc*128 + p, h]
            rnat_keep = {}

            deferred_final = [None]
            # all output rows collected on partition 0; stored once at the end
            out_all = singles.tile([1, bpc * H], FP32, tag="out_all")
            junk_dram = singles.tile(
                [8, 128], BF16, tag="junk_dram", space="DRAM"
            )

            def emit_final(b):
                probs_row, rsum = deferred_final[0]
                # normalized alpha (bf16), in place: probs * (1/sumexp)
                probs_n = probs_row
                nc.vector.tensor_scalar_mul(probs_n, probs_row, rsum[:1, :1])
                n_c = s // 128
                alpha_sb = sm_pool.tile([128, s], BF16, tag="alpha_sb", bufs=1)
                expT = sm_pool.tile([128, n_c], FP32, tag="expT")
                expT_bf = sm_pool.tile([128, n_c], BF16, tag="expT_bf")
                ident_b4 = bass.AP(
                    tensor=ident_bf.tensor,
                    offset=ident_bf.offset,
                    ap=[ident_bf.ap[0], [0, 4], ident_bf.ap[1]],
                )
                for st in range(st_n):
                    # broadcast alpha row to all partitions (K=1 ones matmul)
                    ab_ps = ab_pool.tile([128, 512], FP32, tag="ab_ps", bufs=2)
                    nc.tensor.matmul(
                        ab_ps,
                        lhsT=ones_row,
                        rhs=probs_n[:, st * 512: (st + 1) * 512],
                        start=True,
                        stop=True,
                    )
                    a_blk = alpha_sb[:, st * 512: (st + 1) * 512].rearrange(
                        "p (c sl) -> p c sl", c=4
                    )
                    nc.scalar.copy(a_blk, ab_ps.rearrange(
                        "p (c sl) -> p c sl", c=4
                    ))
                    # alpha^T[p, c] = alpha[c*128 + p]: identity-mask + reduce
                    nc.vector.tensor_mul(a_blk, a_blk, ident_b4)
                    nc.vector.reduce_sum(
                        expT[:, st * 4: (st + 1) * 4].rearrange(
                            "p (c o) -> p c o", o=1
                        ),
                        a_blk,
                        axis=mybir.AxisListType.X,
                    )
                    nc.vector.tensor_copy(
                        expT_bf[:, st * 4: (st + 1) * 4],
                        expT[:, st * 4: (st + 1) * 4],
                    )
                # out[b, :] = sum_c expT[:, c]^T @ rnat[c]
                fin_ps = ab_pool.tile([1, H], FP32, tag="fin_ps", bufs=1)
                for c in range(n_c):
                    r_src = rn0a if (b == 0 and c < 4) else rnat_keep[b]
                    r_off = c * H if not (b == 0 and c < 4) else c * H
                    nc.tensor.matmul(
                        fin_ps,
                        lhsT=expT_bf[:, c: c + 1],
                        rhs=r_src[:, r_off: r_off + H],
                        start=(c == 0),
                        stop=(c == n_c - 1),
                    )
                nc.vector.tensor_copy(out_all[:, b * H: (b + 1) * H], fin_ps)

            # ---- main loop ----
            for b in range(bpc):
                scores_row = sm_pool.tile([1, s], FP32, tag="scores_row")
                # ONE load + cast fp32->bf16 (SWDGE) per batch: natural-layout
                # tile [s_part, (chunk h)], chunk = global s//128
                if b == 0:
                    rnat = rnat0
                else:
                    rnat = keep.tile(
                        [128, 4 * st_n * H], BF16, tag="rn", bufs=3, name=f"rn{b}"
                    )
                    nc.gpsimd.dma_start(
                        out=rnat.rearrange("p (t h) -> p t h", t=4 * st_n),
                        in_=sent[b].rearrange("(t p) h -> p t h", p=128),
                    )
                rnat_keep[b] = rnat
                for st in range(st_n):
                    # transpose via DMA xbar: ONE call per (b, st).
                    # in [128s, 2048f=(sc,h)] -> out 3D [p=f%128, j=f//128, s]
                    rT_raw = rt_pool.tile([128, HC * 512], BF16, tag="rT_raw", bufs=3)
                    tr_src = rn0a if (b == 0 and st == 0) else rnat
                    tr_off = 0 if (b == 0 and st == 0) else st * 4 * H
                    nc.sync.dma_start_transpose(
                        out=rT_raw.rearrange("p (j sl) -> p j sl", j=16),
                        in_=tr_src[:, tr_off: tr_off + 4 * H],
                    )
                    rT_blk = rT_raw.rearrange(
                        "p (sc hc sl) -> p sc hc sl", sc=4, hc=HC
                    )
                    # WY^T[k, s] + tanh
                    tanh_t = th_pool.tile([128, KC * 512], BF16, tag="tanh_t")
                    for kc in range(KC):
                        wy = wy_pool.tile([128, 512], FP32, tag="wy", bufs=3)
                        for hc in range(HC):
                            nc.tensor.matmul(
                                wy,
                                lhsT=w_bf[:, hc * H + kc * 128: hc * H + (kc + 1) * 128],
                                rhs=rT_blk[:, :, hc, :],
                                start=(hc == 0),
                                stop=(hc == HC - 1),
                            )
                        nc.scalar.activation(
                            tanh_t[:, kc * 512: (kc + 1) * 512],
                            wy,
                            mybir.ActivationFunctionType.Tanh,
                            bias=wrT[:, kc * bpc + b: kc * bpc + b + 1],
                            scale=1.0,
                        )
                    # scores[s] = sum_k ctx[k] tanh[k, s]: pre-reduce the
                    # kc chunks on DVE (ctx as per-partition scalar), then a
                    # single ones-column matmul sums over partitions
                    g_t = th_pool.tile([128, 512], BF16, tag="g_t")
                    g_tmp = th_pool.tile([128, 512], BF16, tag="g_tmp")
                    nc.vector.tensor_scalar_mul(
                        g_t, tanh_t[:, 0: 512], ctxT_f32[:, 0: 1]
                    )
                    for kc in range(1, KC):
                        nc.vector.tensor_scalar_mul(
                            g_tmp,
                            tanh_t[:, kc * 512: (kc + 1) * 512],
                            ctxT_f32[:, kc: kc + 1],
                        )
                        nc.vector.tensor_add(g_t, g_t, g_tmp)
                    sc_ps = sc_pool.tile([1, 512], FP32, tag="sc_ps")
                    nc.tensor.matmul(
                        sc_ps, lhsT=ones_col, rhs=g_t, start=True, stop=True
                    )
                    nc.vector.tensor_copy(
                        scores_row[:, st * 512: (st + 1) * 512], sc_ps
                    )
                    # interleave previous batch's final phase into this
                    # batch's WY phase so no engine stalls on the softmax
                    if st == 1 and deferred_final[0] is not None:
                        emit_final(b - 1)
                # softmax (no max subtraction: |scores| <= ||ctx||_1)
                probs_row = sm_pool.tile([1, s], BF16, tag="probs_row")
                sumexp = sm_pool.tile([1, 1], FP32, tag="sumexp")
                nc.scalar.activation(
                    probs_row,
                    scores_row,
                    mybir.ActivationFunctionType.Exp,
                    accum_out=sumexp,
                )
                rsum = sm_pool.tile([1, 1], FP32, tag="rsum")
                nc.vector.reciprocal(rsum, sumexp)
                deferred_final[0] = (probs_row, rsum)
            emit_final(bpc - 1)

            # scheduler-only fence, then a junk load to absorb the xbar
            # copy->transpose transition after the last transpose, then the
            # single output store (which then carries only its DVE wait)
            tc.no_sync_barrier()
            junk_sb = singles.tile([1, 128], BF16, tag="junk_sb")
            nc.gpsimd.dma_start(out=junk_sb, in_=junk_dram[0:1, :])
            nc.gpsimd.dma_start(out=out[:, :], in_=out_all)

    nc.compile()
    return nc


def _get_nc(bpc, s):
    key = (bpc, s)
    if key not in _cache:
        _cache[key] = _build_nc(bpc, s)
    return _cache[key]


def _run(sent_bmajor, mean_sent, W, W_h, context, ncores, bpc, s, **kw):
    nc = _get_nc(bpc, s)
    in_maps = []
    for c in range(ncores):
        in_maps.append({
            "sent": np.ascontiguousarray(sent_bmajor[c * bpc: (c + 1) * bpc]),
            "mean_sent": np.ascontiguousarray(mean_sent[c * bpc: (c + 1) * bpc]),
            "w": W,
            "wh": W_h,
            "ctxv": context,
        })
    res = bass_utils.run_bass_kernel_spmd(nc, in_maps, core_ids=list(range(ncores)), **kw)
    outs = np.concatenate([res.results[c]["out"] for c in range(ncores)], axis=0)
    return outs, res


def kernel(sent_batch, mean_sent_batch, batch_mask, W, W_h, context):
    sent_batch = np.asarray(sent_batch, dtype=np.float32)
    batch_mask = np.asarray(batch_mask, dtype=np.float32)
    mean_sent_batch = np.ascontiguousarray(np.asarray(mean_sent_batch, dtype=np.float32))
    W = np.ascontiguousarray(np.asarray(W, dtype=np.float32))
    W_h = np.ascontiguousarray(np.asarray(W_h, dtype=np.float32))
    context = np.ascontiguousarray(np.asarray(context, dtype=np.float32))

    if not np.all(batch_mask == 1.0):
        # general-correctness slow path; the mask is all-ones in this problem
        sent_batch = sent_batch * batch_mask[:, :, None]
    # batch-major contiguous for per-core contiguous shards
    sent_bmajor = np.ascontiguousarray(sent_batch.transpose(1, 0, 2))

    trace = bool(int(os.environ.get("KERNEL_TRACE", "0")))
    outs, res = _run(
        sent_bmajor, mean_sent_batch, W, W_h, context,
        NCORES, BPC, S, trace=trace,
    )
    kernel.last_results = res
    return outs.astype(np.float32)


kernel.last_results = None
